# revision 1
# baseline (speedup 1.0000x reference)
"""BrainGCN on 8 Trainium2 NeuronCores (Bass/Tile, SPMD).

kernel(**inputs) takes the FULL unsharded inputs and returns the full (G,)
output.  Internally:

Sharding: N nodes in 8 contiguous shards (SH=N/8); each edge is assigned to
the core owning its dst node, grouped into 128-node dst windows; per-window
tile counts are equalized across cores so one program runs SPMD on all 8
cores.  Self-loops are excluded from the gathered edge stream; their
contribution is applied per window as a diag(dinv) matmul against locally
retained rows (saves ~6% of the serialized gather calls).

Math: norm_e = dinv[src]*w_e*dinv[dst].  With w'_e = w_e*dinv[src_e]:
  L1: out1[d] = dinv[d] * (sum_e w'_e * x[src_e]) @ (W1*bns1) + c1; relu.
      The aggregation runs in IN_DIM space; x[src_e] is host-pre-gathered
      (pure input data movement), so layer 1 needs no device gathers.  The
      per-edge dinv[src] comes from host-pre-gathered per-node weight-slot
      rows that the device reduces + rsqrts (all FLOPs stay on device).
  L2: t2 = h1 @ (W2*bns2) per own shard; AllGather -> full table;
      out2[d] = dinv[d] * sum_e w'_e * t2[src_e] + c2; relu.
      t2[src_e] is gathered per 128-edge tile with indirect_dma_start using
      a [128,1] offset (the only form this stack executes correctly).
Scatter-add = one-hot matmul into PSUM windows; one-hot built on DVE as
(iota == dloc) * w'.  Pooling: one-hot(batch) matmul accumulated over all
windows plus a ones column for counts, AllReduce, tiny MLP head replicated
on every core; core 0's output is returned.
"""
import math
from contextlib import ExitStack

import numpy as np

import concourse.bass as bass
import concourse.bacc as bacc
import concourse.tile as tile
import concourse.mybir as mybir
from concourse.masks import make_identity
from concourse.bass_utils import run_bass_kernel_spmd

F32 = mybir.dt.float32
I32 = mybir.dt.int32
AL = mybir.AluOpType
ACTF = mybir.ActivationFunctionType

N_CORES = 8
WIN = 128
EPS = 1e-5


def _prep_inputs(inputs: dict):
    x = np.asarray(inputs["x"], np.float32)
    ei = np.asarray(inputs["edge_index"])
    ew = np.asarray(inputs["edge_weight"], np.float32)
    batch = np.asarray(inputs["batch"]).astype(np.int64)
    N, IN_DIM = x.shape
    HID = np.asarray(inputs["W1"]).shape[1]
    assert N % N_CORES == 0
    SH = N // N_CORES
    NWIN = math.ceil(SH / WIN)
    PADN = NWIN * WIN

    # degree includes self-loop weight 1; the edge stream excludes self-loops
    # (their contribution is applied on-device from locally retained rows).
    srcA = np.concatenate([np.asarray(ei[0]), np.arange(N)]).astype(np.int64)
    dstA = np.concatenate([np.asarray(ei[1]), np.arange(N)]).astype(np.int64)
    wA = np.concatenate([ew, np.ones(N, np.float32)]).astype(np.float32)
    orderA = np.argsort(dstA, kind="stable")
    dsA, wsA = dstA[orderA], wA[orderA]
    countsA = np.bincount(dstA, minlength=N)
    DSLOT = int(countsA.max())
    rowptrA = np.zeros(N + 1, np.int64)
    np.cumsum(countsA, out=rowptrA[1:])
    wdeg_full = np.zeros((N, DSLOT), np.float32)
    slotA = np.arange(len(dsA)) - rowptrA[dsA]
    wdeg_full[dsA, slotA] = wsA

    src = np.asarray(ei[0]).astype(np.int64)
    dst = np.asarray(ei[1]).astype(np.int64)
    w = ew.astype(np.float32)
    order = np.argsort(dst, kind="stable")
    ds, ss, ws = dst[order], src[order], w[order]
    counts = np.bincount(dst, minlength=N)
    rowptr = np.zeros(N + 1, np.int64)
    np.cumsum(counts, out=rowptr[1:])

    cnt = np.zeros((N_CORES, NWIN), np.int64)
    seg = {}
    for c in range(N_CORES):
        for j in range(NWIN):
            lo = c * SH + j * WIN
            hi = min(c * SH + (j + 1) * WIN, (c + 1) * SH)
            a, b = int(rowptr[lo]), int(rowptr[hi])
            cnt[c, j] = b - a
            seg[(c, j)] = (lo, a, b)
    tiles = np.maximum(1, -(-cnt.max(axis=0) // 128)).astype(np.int64)
    TT = int(tiles.sum())
    colptr = np.zeros(NWIN + 1, np.int64)
    np.cumsum(tiles, out=colptr[1:])

    esrc = np.zeros((N_CORES, 128, TT), np.int32)
    edloc = np.full((N_CORES, 128, TT), 999.0, np.float32)
    ewt = np.zeros((N_CORES, 128, TT), np.float32)
    xg = np.zeros((N_CORES, 128, TT * IN_DIM), np.float32)
    wde = np.zeros((N_CORES, 128, TT * DSLOT), np.float32)
    for c in range(N_CORES):
        for j in range(NWIN):
            lo, a, b = seg[(c, j)]
            n = b - a
            tj = int(tiles[j])
            cap = tj * 128
            s_pad = np.zeros(cap, np.int64)
            d_pad = np.full(cap, 999.0, np.float32)
            w_pad = np.zeros(cap, np.float32)
            s_pad[:n] = ss[a:b]
            d_pad[:n] = (ds[a:b] - lo).astype(np.float32)
            w_pad[:n] = ws[a:b]
            c0 = int(colptr[j])
            esrc[c, :, c0:c0 + tj] = s_pad.reshape(tj, 128).T.astype(np.int32)
            edloc[c, :, c0:c0 + tj] = d_pad.reshape(tj, 128).T
            ewt[c, :, c0:c0 + tj] = w_pad.reshape(tj, 128).T
            xs = x[s_pad]
            xs[n:] = 0.0
            wd = wdeg_full[s_pad].copy()
            wd[n:] = 0.0
            wd[n:, 0] = 1.0  # pad edges: deg=1 keeps rsqrt finite
            xg[c, :, c0 * IN_DIM:(c0 + tj) * IN_DIM] = (
                xs.reshape(tj, 128, IN_DIM).transpose(1, 0, 2)
                .reshape(128, tj * IN_DIM))
            wde[c, :, c0 * DSLOT:(c0 + tj) * DSLOT] = (
                wd.reshape(tj, 128, DSLOT).transpose(1, 0, 2)
                .reshape(128, tj * DSLOT))

    def win_major(a2d):
        S = a2d.shape[1]
        return np.ascontiguousarray(
            a2d.reshape(NWIN, WIN, S).transpose(1, 0, 2).reshape(WIN, NWIN * S))

    in_maps = []
    for c in range(N_CORES):
        wc = np.zeros((PADN, DSLOT), np.float32)
        wc[:SH] = wdeg_full[c * SH:(c + 1) * SH]
        wc[SH:, 0] = 1.0
        bv = np.full((PADN, 1), 999.0, np.float32)
        bv[:SH, 0] = batch[c * SH:(c + 1) * SH].astype(np.float32)
        xo = np.zeros((PADN, IN_DIM), np.float32)
        xo[:SH] = x[c * SH:(c + 1) * SH]
        HIDv = HID
        in_maps.append({
            "xnm": win_major(xo),
            "esrc": esrc[c], "edloc": edloc[c], "ewt": ewt[c],
            "xg": xg[c], "wde": wde[c],
            "wdeg": win_major(wc), "batchv": win_major(bv),
            "W1": np.asarray(inputs["W1"], np.float32),
            "W2": np.asarray(inputs["W2"], np.float32),
            "g1": np.asarray(inputs["bn1_gamma"], np.float32).reshape(1, HIDv),
            "be1": np.asarray(inputs["bn1_beta"], np.float32).reshape(1, HIDv),
            "m1": np.asarray(inputs["bn1_mean"], np.float32).reshape(1, HIDv),
            "v1": np.asarray(inputs["bn1_var"], np.float32).reshape(1, HIDv),
            "b1": np.asarray(inputs["b1"], np.float32).reshape(1, HIDv),
            "g2": np.asarray(inputs["bn2_gamma"], np.float32).reshape(1, HIDv),
            "be2": np.asarray(inputs["bn2_beta"], np.float32).reshape(1, HIDv),
            "m2": np.asarray(inputs["bn2_mean"], np.float32).reshape(1, HIDv),
            "v2": np.asarray(inputs["bn2_var"], np.float32).reshape(1, HIDv),
            "b2": np.asarray(inputs["b2"], np.float32).reshape(1, HIDv),
            "lin1W": np.asarray(inputs["lin1_W"], np.float32),
            "lin1b": np.asarray(inputs["lin1_b"], np.float32).reshape(-1, 1),
            "lin2W": np.asarray(inputs["lin2_W"], np.float32),
            "lin2b": np.asarray(inputs["lin2_b"], np.float32).reshape(1, 1),
        })

    meta = dict(N=N, G=128, IN_DIM=IN_DIM, HID=HID, SH=SH, NWIN=NWIN,
                DSLOT=DSLOT, TT=TT, tiles=[int(t) for t in tiles])
    return in_maps, meta


def _build_nc(meta):
    N, IN_DIM, HID = meta["N"], meta["IN_DIM"], meta["HID"]
    SH, NWIN, DSLOT, TT = meta["SH"], meta["NWIN"], meta["DSLOT"], meta["TT"]
    tiles = meta["tiles"]
    H2 = HID // 2

    nc = bacc.Bacc("TRN2", target_bir_lowering=False, debug=False,
                   num_devices=N_CORES)
    d_esrc = nc.dram_tensor("esrc", [128, TT], I32, kind="ExternalInput")
    d_edloc = nc.dram_tensor("edloc", [128, TT], F32, kind="ExternalInput")
    d_ewt = nc.dram_tensor("ewt", [128, TT], F32, kind="ExternalInput")
    d_xg = nc.dram_tensor("xg", [128, TT * IN_DIM], F32, kind="ExternalInput")
    d_wde = nc.dram_tensor("wde", [128, TT * DSLOT], F32, kind="ExternalInput")
    d_wdeg = nc.dram_tensor("wdeg", [128, NWIN * DSLOT], F32, kind="ExternalInput")
    d_batch = nc.dram_tensor("batchv", [128, NWIN], F32, kind="ExternalInput")
    d_xnm = nc.dram_tensor("xnm", [128, NWIN * IN_DIM], F32, kind="ExternalInput")
    d_W1 = nc.dram_tensor("W1", [IN_DIM, HID], F32, kind="ExternalInput")
    d_W2 = nc.dram_tensor("W2", [HID, HID], F32, kind="ExternalInput")
    bn_names = ["g1", "be1", "m1", "v1", "b1", "g2", "be2", "m2", "v2", "b2"]
    d_bn = {k: nc.dram_tensor(k, [1, HID], F32, kind="ExternalInput")
            for k in bn_names}
    d_lin1W = nc.dram_tensor("lin1W", [HID, H2], F32, kind="ExternalInput")
    d_lin1b = nc.dram_tensor("lin1b", [H2, 1], F32, kind="ExternalInput")
    d_lin2W = nc.dram_tensor("lin2W", [H2, 1], F32, kind="ExternalInput")
    d_lin2b = nc.dram_tensor("lin2b", [1, 1], F32, kind="ExternalInput")
    d_out = nc.dram_tensor("out", [1, 128], F32, kind="ExternalOutput")

    rg = [list(range(N_CORES))]

    with tile.TileContext(nc) as tc, ExitStack() as ctx:
        constp = ctx.enter_context(tc.tile_pool(name="const", bufs=1))
        metap = ctx.enter_context(tc.tile_pool(name="meta", bufs=1))
        wdep = ctx.enter_context(tc.tile_pool(name="wdep", bufs=2))
        msgsp = ctx.enter_context(tc.tile_pool(name="msgs", bufs=3))
        ohp = ctx.enter_context(tc.tile_pool(name="oh", bufs=6))
        epp = ctx.enter_context(tc.tile_pool(name="ep", bufs=3))
        vecp = ctx.enter_context(tc.tile_pool(name="vec", bufs=1))
        psA = ctx.enter_context(tc.tile_pool(name="psA", bufs=2, space="PSUM"))
        ps5 = ctx.enter_context(tc.tile_pool(name="ps5", bufs=2, space="PSUM"))
        psB = ctx.enter_context(tc.tile_pool(name="psB", bufs=2, space="PSUM"))
        psPool = ctx.enter_context(tc.tile_pool(name="psP", bufs=1, space="PSUM"))
        dram = ctx.enter_context(tc.tile_pool(name="dram", bufs=1, space="DRAM"))

        iota = constp.tile([128, 128], F32)
        nc.gpsimd.iota(iota[:], pattern=[[1, 128]], base=0, channel_multiplier=0,
                       allow_small_or_imprecise_dtypes=True)
        ident = constp.tile([128, 128], F32)
        make_identity(nc, ident[:])
        ones1 = constp.tile([1, 128], F32)
        nc.vector.memset(ones1[:], 1.0)

        sb_esrc = metap.tile([128, TT], I32)
        sb_edloc = metap.tile([128, TT], F32)
        sb_ewt = metap.tile([128, TT], F32)
        sb_batch = metap.tile([128, NWIN], F32)
        sb_xg = metap.tile([128, TT * IN_DIM], F32)
        sb_xnm = metap.tile([128, NWIN * IN_DIM], F32)
        nc.sync.dma_start(sb_xnm[:], d_xnm.ap())
        t2keep = metap.tile([128, NWIN * HID], F32)
        nc.sync.dma_start(sb_esrc[:], d_esrc.ap())
        nc.sync.dma_start(sb_edloc[:], d_edloc.ap())
        nc.sync.dma_start(sb_ewt[:], d_ewt.ap())
        nc.sync.dma_start(sb_batch[:], d_batch.ap())
        nc.sync.dma_start(sb_xg[:], d_xg.ap())
        sb_W1 = constp.tile([IN_DIM, HID], F32)
        sb_W2 = constp.tile([HID, HID], F32)
        nc.sync.dma_start(sb_W1[:], d_W1.ap())
        nc.sync.dma_start(sb_W2[:], d_W2.ap())
        sb_bn = {}
        for k in bn_names:
            sb_bn[k] = vecp.tile([1, HID], F32, tag=k, name="sb_" + k)
            nc.sync.dma_start(sb_bn[k][:], d_bn[k].ap())
        sb_lin1W = constp.tile([HID, H2], F32)
        sb_lin1b = constp.tile([H2, 1], F32)
        sb_lin2W = constp.tile([H2, 1], F32)
        sb_lin2b = constp.tile([1, 1], F32)
        nc.sync.dma_start(sb_lin1W[:], d_lin1W.ap())
        nc.sync.dma_start(sb_lin1b[:], d_lin1b.ap())
        nc.sync.dma_start(sb_lin2W[:], d_lin2W.ap())
        nc.sync.dma_start(sb_lin2b[:], d_lin2b.ap())

        # BN folds: bns = gamma*rsqrt(var+eps); c = bns*(b - mean) + beta
        def bn_fold(g, be, m, v, b):
            bns = vecp.tile([1, HID], F32, tag="bns" + g, name="bns" + g)
            nc.vector.tensor_scalar(out=bns[:], in0=sb_bn[v][:], scalar1=EPS,
                                    scalar2=None, op0=AL.add)
            nc.scalar.activation(bns[:], bns[:], ACTF.Sqrt)
            nc.vector.reciprocal(bns[:], bns[:])
            nc.vector.tensor_tensor(out=bns[:], in0=bns[:], in1=sb_bn[g][:],
                                    op=AL.mult)
            cc = vecp.tile([1, HID], F32, tag="c" + g, name="c" + g)
            nc.vector.tensor_tensor(out=cc[:], in0=sb_bn[b][:], in1=sb_bn[m][:],
                                    op=AL.subtract)
            nc.vector.tensor_tensor(out=cc[:], in0=cc[:], in1=bns[:], op=AL.mult)
            nc.vector.tensor_tensor(out=cc[:], in0=cc[:], in1=sb_bn[be][:],
                                    op=AL.add)
            return bns, cc

        bns1, c1v = bn_fold("g1", "be1", "m1", "v1", "b1")
        bns2, c2v = bn_fold("g2", "be2", "m2", "v2", "b2")

        def bcast128(vec, tag):
            ps = psB.tile([128, HID], F32, tag="B", name="bc" + tag)
            nc.tensor.matmul(out=ps[:], lhsT=ones1[:], rhs=vec[:],
                             start=True, stop=True)
            sb = constp.tile([128, HID], F32, tag=tag, name="sb" + tag)
            nc.vector.tensor_copy(sb[:], ps[:])
            return sb

        c1_b = bcast128(c1v, "c1b")
        c2_b = bcast128(c2v, "c2b")

        def wfold(sb_W, bns, parts, tag):
            one_r = constp.tile([1, parts], F32, tag="oner" + tag,
                                name="oner" + tag)
            nc.vector.memset(one_r[:], 1.0)
            ps = psB.tile([parts, HID], F32, tag="B", name="wf" + tag)
            nc.tensor.matmul(out=ps[:], lhsT=one_r[:], rhs=bns[:],
                             start=True, stop=True)
            wp = constp.tile([parts, HID], F32, tag="wp" + tag, name="wp" + tag)
            nc.vector.tensor_tensor(out=wp[:], in0=sb_W[:], in1=ps[:], op=AL.mult)
            return wp

        W1p = wfold(sb_W1, bns1, IN_DIM, "1")
        W2p = wfold(sb_W2, bns2, HID, "2")

        # own-shard degree -> dinv_d [128, NWIN]
        sb_wdeg = metap.tile([128, NWIN * DSLOT], F32)
        nc.sync.dma_start(sb_wdeg[:], d_wdeg.ap())
        deg = constp.tile([128, NWIN], F32)
        nc.vector.tensor_reduce(
            out=deg[:].rearrange("p (j s) -> p j s", s=1),
            in_=sb_wdeg[:].rearrange("p (j s) -> p j s", s=DSLOT),
            op=AL.add, axis=mybir.AxisListType.X)
        dinv = constp.tile([128, NWIN], F32)
        nc.scalar.activation(dinv[:], deg[:], ACTF.Sqrt)
        nc.vector.reciprocal(dinv[:], dinv[:])

        # per-edge w' = ewt * rsqrt(deg[src]) via chunked wde reduction
        wprime = constp.tile([128, TT], F32)
        CH = 128
        for c0 in range(0, TT, CH):
            cw = min(CH, TT - c0)
            wchunk = wdep.tile([128, CH * DSLOT], F32, tag="wde", name="wchunk")
            nc.sync.dma_start(wchunk[:, :cw * DSLOT],
                              d_wde.ap()[:, c0 * DSLOT:(c0 + cw) * DSLOT])
            nc.vector.tensor_reduce(
                out=wprime[:, c0:c0 + cw].rearrange("p (j s) -> p j s", s=1),
                in_=wchunk[:, :cw * DSLOT].rearrange("p (j s) -> p j s", s=DSLOT),
                op=AL.add, axis=mybir.AxisListType.X)
            nc.scalar.activation(wprime[:, c0:c0 + cw], wprime[:, c0:c0 + cw],
                                 ACTF.Sqrt)
            nc.vector.reciprocal(wprime[:, c0:c0 + cw], wprime[:, c0:c0 + cw])
            nc.vector.tensor_tensor(out=wprime[:, c0:c0 + cw],
                                    in0=wprime[:, c0:c0 + cw],
                                    in1=sb_ewt[:, c0:c0 + cw], op=AL.mult)

        t2_sh = dram.tile([SH, HID], F32)
        t2_full = dram.tile([N, HID], F32)

        def build_diag(j):
            dg = ohp.tile([128, 128], F32, tag="oh", name="dg")
            nc.vector.tensor_scalar(
                out=dg[:], in0=ident[:], scalar1=dinv[:, j:j + 1],
                scalar2=None, op0=AL.mult)
            return dg

        def build_onehot(col):
            oh = ohp.tile([128, 128], F32, tag="oh", name="oh")
            nc.vector.tensor_scalar(
                out=oh[:], in0=iota[:],
                scalar1=sb_edloc[:, col:col + 1],
                scalar2=wprime[:, col:col + 1],
                op0=AL.is_equal, op1=AL.mult)
            return oh

        colptr = np.concatenate([[0], np.cumsum(tiles)]).astype(int)

        # L1: aggregate host-gathered x in IN_DIM space, then project
        for j in range(NWIN):
            Tj = tiles[j]
            col = int(colptr[j])
            wlen = min(WIN, SH - j * WIN)
            acc5 = ps5.tile([IN_DIM, 128], F32, tag="acc5", name="acc5")
            for t in range(Tj):
                oh = build_onehot(col + t)
                nc.tensor.matmul(
                    out=acc5[:],
                    lhsT=sb_xg[:, (col + t) * IN_DIM:(col + t + 1) * IN_DIM],
                    rhs=oh[:], start=(t == 0), stop=False)
            dg1 = build_diag(j)
            nc.tensor.matmul(
                out=acc5[:],
                lhsT=sb_xnm[:, j * IN_DIM:(j + 1) * IN_DIM],
                rhs=dg1[:], start=False, stop=True)
            agg5 = epp.tile([IN_DIM, 128], F32, tag="agg5", name="agg5")
            nc.vector.tensor_copy(agg5[:], acc5[:])
            ps1 = psB.tile([128, HID], F32, tag="B", name="ps1")
            nc.tensor.matmul(out=ps1[:], lhsT=agg5[:], rhs=W1p[:],
                             start=True, stop=True)
            h1 = epp.tile([128, HID], F32, tag="h1", name="h1")
            nc.vector.tensor_scalar(out=h1[:], in0=ps1[:],
                                    scalar1=dinv[:, j:j + 1], scalar2=None,
                                    op0=AL.mult)
            nc.vector.tensor_tensor(out=h1[:], in0=h1[:], in1=c1_b[:], op=AL.add)
            nc.scalar.activation(h1[:], h1[:], ACTF.Relu)
            pT = psB.tile([HID, 128], F32, tag="B", name="pT")
            nc.tensor.transpose(out=pT[:], in_=h1[:], identity=ident[:])
            h1T = epp.tile([HID, 128], F32, tag="h1T", name="h1T")
            nc.vector.tensor_copy(h1T[:], pT[:])
            ps2 = psB.tile([128, HID], F32, tag="B", name="ps2")
            nc.tensor.matmul(out=ps2[:], lhsT=h1T[:], rhs=W2p[:],
                             start=True, stop=True)
            t2r = t2keep[:, j * HID:(j + 1) * HID]
            nc.vector.tensor_copy(t2r, ps2[:])
            nc.sync.dma_start(t2_sh[j * WIN:j * WIN + wlen, :], t2r[:wlen, :])

        nc.gpsimd.collective_compute(
            "AllGather", AL.bypass, replica_groups=rg,
            ins=[t2_sh.opt()], outs=[t2_full.opt()])

        # L2: per-edge 256B gathers + one-hot scatter + pooling
        pool_ps = psPool.tile([128, HID + 1], F32)
        for j in range(NWIN):
            Tj = tiles[j]
            col = int(colptr[j])
            msgs = msgsp.tile([128, Tj * HID], F32, tag="msgs", name="msgs")
            for t in range(Tj):
                nc.gpsimd.indirect_dma_start(
                    out=msgs[:, t * HID:(t + 1) * HID], out_offset=None,
                    in_=t2_full[:],
                    in_offset=bass.IndirectOffsetOnAxis(
                        ap=sb_esrc[:, col + t:col + t + 1], axis=0))
            acc = psA.tile([128, HID], F32, tag="acc", name="acc")
            for t in range(Tj):
                oh = build_onehot(col + t)
                nc.tensor.matmul(out=acc[:], lhsT=oh[:],
                                 rhs=msgs[:, t * HID:(t + 1) * HID],
                                 start=(t == 0), stop=False)
            dg2 = build_diag(j)
            nc.tensor.matmul(out=acc[:], lhsT=dg2[:],
                             rhs=t2keep[:, j * HID:(j + 1) * HID],
                             start=False, stop=True)
            h2e = epp.tile([128, HID + 1], F32, tag="h2e", name="h2e")
            nc.vector.tensor_scalar(out=h2e[:, :HID], in0=acc[:],
                                    scalar1=dinv[:, j:j + 1], scalar2=None,
                                    op0=AL.mult)
            nc.vector.tensor_tensor(out=h2e[:, :HID], in0=h2e[:, :HID],
                                    in1=c2_b[:], op=AL.add)
            nc.scalar.activation(h2e[:, :HID], h2e[:, :HID], ACTF.Relu)
            nc.vector.memset(h2e[:, HID:], 1.0)
            ohb = ohp.tile([128, 128], F32, tag="ohb", name="ohb")
            nc.vector.tensor_scalar(out=ohb[:], in0=iota[:],
                                    scalar1=sb_batch[:, j:j + 1], scalar2=None,
                                    op0=AL.is_equal)
            nc.tensor.matmul(out=pool_ps[:], lhsT=ohb[:], rhs=h2e[:],
                             start=(j == 0), stop=(j == NWIN - 1),
                             skip_group_check=True)

        pool_sb = epp.tile([128, HID + 1], F32, tag="poolsb", name="pool_sb")
        nc.vector.tensor_copy(pool_sb[:], pool_ps[:])
        ar_in = dram.tile([128, HID + 1], F32)
        ar_out = dram.tile([128, HID + 1], F32)
        nc.sync.dma_start(ar_in[:], pool_sb[:])
        nc.gpsimd.collective_compute(
            "AllReduce", AL.add, replica_groups=rg,
            ins=[ar_in.opt()], outs=[ar_out.opt()])
        sums = epp.tile([128, HID + 1], F32, tag="sums", name="sums")
        nc.sync.dma_start(sums[:], ar_out[:])

        cntc = epp.tile([128, 1], F32, tag="cnt", name="cntc")
        nc.vector.tensor_scalar(out=cntc[:], in0=sums[:, HID:HID + 1],
                                scalar1=1.0, scalar2=None, op0=AL.max)
        rc = epp.tile([128, 1], F32, tag="rc", name="rc")
        nc.vector.reciprocal(rc[:], cntc[:])
        pooled = epp.tile([128, HID], F32, tag="pooled", name="pooled")
        nc.vector.tensor_scalar(out=pooled[:], in0=sums[:, :HID],
                                scalar1=rc[:, :1], scalar2=None, op0=AL.mult)
        pT2 = psB.tile([HID, 128], F32, tag="B", name="pT2")
        nc.tensor.transpose(out=pT2[:], in_=pooled[:], identity=ident[:])
        pooledT = epp.tile([HID, 128], F32, tag="pooledT", name="pooledT")
        nc.vector.tensor_copy(pooledT[:], pT2[:])
        zps = psB.tile([H2, 128], F32, tag="B", name="zps")
        nc.tensor.matmul(out=zps[:], lhsT=sb_lin1W[:], rhs=pooledT[:],
                         start=True, stop=True)
        zT = epp.tile([H2, 128], F32, tag="zT", name="zT")
        nc.scalar.activation(zT[:], zps[:], ACTF.Relu, bias=sb_lin1b[:, :1])
        ops = psB.tile([1, 128], F32, tag="B", name="ops")
        nc.tensor.matmul(out=ops[:], lhsT=sb_lin2W[:], rhs=zT[:],
                         start=True, stop=True)
        outsb = epp.tile([1, 128], F32, tag="outsb", name="outsb")
        nc.vector.tensor_scalar(out=outsb[:], in0=ops[:],
                                scalar1=sb_lin2b[:, :1], scalar2=None, op0=AL.add)
        nc.sync.dma_start(d_out.ap(), outsb[:])

    nc.compile()
    return nc


_CACHE = {}


def kernel(**inputs) -> np.ndarray:
    in_maps, meta = _prep_inputs(inputs)
    key = (meta["N"], meta["TT"], meta["DSLOT"], tuple(meta["tiles"]))
    if key not in _CACHE:
        _CACHE[key] = _build_nc(meta)
    nc = _CACHE[key]
    res = run_bass_kernel_spmd(nc, in_maps, core_ids=list(range(N_CORES)))
    out = np.asarray(res.results[0]["out"], np.float32).reshape(-1)
    return out[:meta["G"]].copy()

